# revision 16
# baseline (speedup 1.0000x reference)
"""PoPE attention Trainium2 kernel, 8-core tensor-parallel (2 heads/core).

Self-contained: hardcodes shapes B=1, S=2048, HID=2048, NH=16, HD=128.

Math (per reference):
  q/k/v = X @ w{q,k,v}.T, split into 16 heads of dim 128
  mu_{q,k} = softplus(q/k)
  q_polar = mu_q * (cos/sin)(pos*invfreq);  k uses angles + clipped bias
  scores  = (q_re.k_re + q_im.k_im)/sqrt(128) + causal_mask
  out     = softmax(scores) @ v;  final = out @ wo.T

Sharding: heads 2c,2c+1 on core c (wq/wk/wv column-sharded). The output
projection is row-sharded after on-device chunked AllGathers of the
per-core attention outputs; each core computes a 256-column slice of the
final output (transposed layout), gathered and transposed on host.

Device layout is feature-major ("transposed"): activations live as
[d, s] so every matmul contracts along partitions with zero on-device
transposes. Scores are computed as E[s', t] so softmax's sum reduction
is a ones-vector matmul and E feeds the A@V matmul directly.

All DRAM inputs are pre-arranged on the host into the exact SBUF layout
(partition-major) so every load is a contiguous DMA, and the startup
loads are spread across four engine queues so issue and transfer
overlap. The k-side rotation bias is folded into per-head cos/sin
tables on the host. The kernel is emitted as one software-pipelined
stream: projections for chunk j+1 and the output projection for chunk
j-1 are interleaved between the attention blocks of chunk j, each
block's first two score tiles lead the interleaved projection
activations on the ACT queue, and the normalize/gather tail of each
block is deferred past the next block's head so the PE never waits on
the reciprocal chain. The causal mask is a multiplicative 0/1 triangle
applied to E after the exp; diagonal-tile AV/rowsum matmuls run on
partial free ranges. The last chunk's AllGathers are per-head and its
output projection consumes the first head's gather while the second is
still in flight.
"""

import math
import sys
import types

import numpy as np
import ml_dtypes

import concourse.bass as bass
import concourse.mybir as mybir
import concourse.tile as tile
from concourse.bass_utils import run_bass_kernel_spmd

# ---------------------------------------------------------------- constants
B, S, HID = 1, 2048, 2048
NH, HD = 16, 128
BASE = 10000.0
N_CORES = 8
HPC = NH // N_CORES          # heads per core = 2
DPC = HPC * HD               # head dims per core = 256
P = 128                      # partitions
KO = HID // P                # 16 k-subtiles
NCH = S // 512               # 4 free-dim chunks of 512
SQ = S // P                  # 16 s'-tiles of 128
BF16 = mybir.dt.bfloat16
F32 = mybir.dt.float32
AF = mybir.ActivationFunctionType
ALU = mybir.AluOpType
ISQ = 1.0 / math.sqrt(HD)


def _install_ntff_hook():
    """Bare agent image lacks antenv.axon_hooks; synthesize it from the boot
    module's ctypes NTFF hook so run_bass_kernel_spmd(trace=True) works."""
    if "antenv.axon_hooks" in sys.modules:
        return
    try:
        from trn_agent_boot.trn_boot import _ntff_profile_via_ctypes
        hook = _ntff_profile_via_ctypes("/opt/axon/libaxon_pjrt.so")
    except Exception:
        hook = None
    mod = types.ModuleType("antenv.axon_hooks")
    mod.get_axon_ntff_profile_hook = lambda: hook
    mod.set_axon_ntff_profile_hook = lambda h: None
    sys.modules["antenv.axon_hooks"] = mod


_install_ntff_hook()

_TPB_ENGINES = (
    mybir.EngineType.PE,
    mybir.EngineType.Activation,
    mybir.EngineType.DVE,
    mybir.EngineType.Pool,
    mybir.EngineType.SP,
)


class SplitDrainTileContext(tile.TileContext):
    """This walrus build allows at most ONE sem wait per TPB instruction.
    Legalize: move extra waits onto single-wait NOPs emitted just before the
    instruction on the same engine, and split the tail drain the same way."""

    def _split_multiwait(self, insts):
        out = []
        for inst in insts:
            si = getattr(inst, "sync_info", None)
            if (
                si is not None
                and si.on_wait
                and len(si.on_wait) > 1
                and inst.engine in _TPB_ENGINES
            ):
                waits = list(si.on_wait)
                for w in waits[:-1]:
                    out.append(
                        mybir.InstNoOp(
                            name=self.nc.get_next_instruction_name(),
                            sync_info=mybir.SyncInfo(on_wait=[w], on_update=[]),
                            bass_nofuse=True,
                            engine=inst.engine,
                        )
                    )
                si.on_wait = waits[-1:]
            out.append(inst)
        return out

    def _lower_ordered_insts(self, ordered):
        for k in list(ordered.keys()):
            ordered[k] = self._split_multiwait(ordered[k])
        return super()._lower_ordered_insts(ordered)

    def _drain_and_barrier(self, tick_clock, wait_clock):
        from concourse.vector_clock import ScopedClock

        drain_inst = self.nc.sync.drain()
        wait_clock.add_sem_waits(
            drain_inst.ins, ScopedClock({None: tick_clock.global_clock})
        )
        waits = list(drain_inst.ins.sync_info.on_wait or [])
        if len(waits) > 1:
            drain_inst.ins.sync_info.on_wait = waits[:1]
            for w in waits[1:]:
                d2 = self.nc.sync.drain()
                if d2.ins.sync_info is None:
                    d2.ins.sync_info = mybir.SyncInfo(on_wait=[w], on_update=[])
                else:
                    d2.ins.sync_info.on_wait = [w]

        self.nc.all_engine_barrier()
        assert self.sems is not None
        popped = self.nc._tile_sem_poison_stack.pop()
        assert popped is self._sem_poison
        self.nc.clear_and_free_semaphores(list(self.sems.allocated().values()))
        self.nc.all_engine_barrier()


def build_nc():
    nc = bass.Bass("TRN2", target_bir_lowering=False, debug=False,
                   num_devices=N_CORES)

    # all inputs pre-arranged on host to partition-major device layout
    xt_d = nc.dram_tensor("xt", [P, NCH, KO, 512], BF16,
                          kind="ExternalInput").ap()
    wq_d = nc.dram_tensor("wq", [P, KO, DPC], BF16, kind="ExternalInput").ap()
    wk_d = nc.dram_tensor("wk", [P, KO, DPC], BF16, kind="ExternalInput").ap()
    wv_d = nc.dram_tensor("wv", [P, KO, DPC], BF16, kind="ExternalInput").ap()
    cos_d = nc.dram_tensor("cosT", [P, S], BF16, kind="ExternalInput").ap()
    sin_d = nc.dram_tensor("sinT", [P, S], BF16, kind="ExternalInput").ap()
    # per-head k-side tables with the clipped bias folded in
    ck_d = nc.dram_tensor("ckT", [P, HPC, S], BF16, kind="ExternalInput").ap()
    sk_d = nc.dram_tensor("skT", [P, HPC, S], BF16, kind="ExternalInput").ap()
    tri_d = nc.dram_tensor("tri", [P, P], BF16, kind="ExternalInput").ap()
    wo3_d = nc.dram_tensor("wo3", [P, HPC, NH * HD], BF16,
                           kind="ExternalInput").ap()
    out3_d = nc.dram_tensor("out3", [NH * HD, S], BF16,
                            kind="ExternalOutput").ap()

    with SplitDrainTileContext(nc) as tc:
        with tc.tile_pool(name="big", bufs=1) as big, \
             tc.tile_pool(name="wts", bufs=1) as wts, \
             tc.tile_pool(name="tabs", bufs=1) as tabs, \
             tc.tile_pool(name="qk", bufs=2) as qkp, \
             tc.tile_pool(name="mu", bufs=4) as mup, \
             tc.tile_pool(name="ep", bufs=6) as ep, \
             tc.tile_pool(name="sm", bufs=2) as smp, \
             tc.tile_pool(name="ob", bufs=3) as obp, \
             tc.tile_pool(name="ps", bufs=1, space="PSUM") as psp, \
             tc.tile_pool(name="dram", bufs=1, space="DRAM") as dram:

            # ------- loads: SP+ACT queues split the PE-critical path in
            # halves (a single DMA queue sustains only ~150GB/s); gpsimd
            # carries the DVE tables and everything needed later.
            wq_sb = wts.tile([P, KO, DPC], BF16, name="wq_sb")
            xt_sb = big.tile([P, NCH, KO, 512], BF16, tag="big", name="xt_sb")
            wk_sb = wts.tile([P, KO, DPC], BF16, name="wk_sb")
            wo3_sb = wts.tile([P, HPC, NH * HD], BF16, name="wo3_sb")
            nc.sync.dma_start(wq_sb[:, 0:2, :], wq_d[:, 0:2, :])
            nc.scalar.dma_start(xt_sb[:, 0, 0:1, :], xt_d[:, 0, 0:1, :])
            nc.sync.dma_start(xt_sb[:, 0, 1:2, :], xt_d[:, 0, 1:2, :])
            nc.scalar.dma_start(wq_sb[:, 2:4, :], wq_d[:, 2:4, :])
            nc.sync.dma_start(xt_sb[:, 0, 2:3, :], xt_d[:, 0, 2:3, :])
            nc.scalar.dma_start(xt_sb[:, 0, 3:4, :], xt_d[:, 0, 3:4, :])
            nc.sync.dma_start(wq_sb[:, 4:8, :], wq_d[:, 4:8, :])
            nc.sync.dma_start(xt_sb[:, 0, 4:6, :], xt_d[:, 0, 4:6, :])
            nc.scalar.dma_start(xt_sb[:, 0, 6:8, :], xt_d[:, 0, 6:8, :])
            nc.sync.dma_start(wq_sb[:, 8:12, :], wq_d[:, 8:12, :])
            nc.scalar.dma_start(xt_sb[:, 0, 8:10, :], xt_d[:, 0, 8:10, :])
            nc.sync.dma_start(xt_sb[:, 0, 10:12, :], xt_d[:, 0, 10:12, :])
            nc.scalar.dma_start(wq_sb[:, 12:16, :], wq_d[:, 12:16, :])
            nc.sync.dma_start(xt_sb[:, 0, 12:14, :], xt_d[:, 0, 12:14, :])
            nc.scalar.dma_start(xt_sb[:, 0, 14:16, :], xt_d[:, 0, 14:16, :])
            nc.sync.dma_start(wk_sb[:, 0:8, :], wk_d[:, 0:8, :])
            nc.scalar.dma_start(wk_sb[:, 8:16, :], wk_d[:, 8:16, :])

            # gpsimd queue: tables for chunk 0 (DVE-critical), then wv
            cos_sb = tabs.tile([P, S], BF16, name="cos_sb")
            sin_sb = tabs.tile([P, S], BF16, name="sin_sb")
            ck_sb = tabs.tile([P, HPC, S], BF16, name="ck_sb")
            sk_sb = tabs.tile([P, HPC, S], BF16, name="sk_sb")
            tri_sb = tabs.tile([P, P], BF16, name="tri_sb")
            wv_sb = wts.tile([P, KO, DPC], BF16, name="wv_sb")
            nc.gpsimd.dma_start(cos_sb[:, 0:512], cos_d[:, 0:512])
            nc.gpsimd.dma_start(sin_sb[:, 0:512], sin_d[:, 0:512])
            nc.gpsimd.dma_start(ck_sb[:, :, 0:512], ck_d[:, :, 0:512])
            nc.gpsimd.dma_start(sk_sb[:, :, 0:512], sk_d[:, :, 0:512])
            nc.gpsimd.dma_start(tri_sb[:], tri_d[:])
            nc.gpsimd.dma_start(wv_sb[:], wv_d[:])

            # everything later-needed goes sequentially on SP so the ACT
            # and gpsimd queues stay clear for compute/memsets/collectives
            nc.sync.dma_start(xt_sb[:, 1, :, :], xt_d[:, 1, :, :])
            nc.sync.dma_start(cos_sb[:, 512:], cos_d[:, 512:])
            nc.sync.dma_start(sin_sb[:, 512:], sin_d[:, 512:])
            nc.sync.dma_start(ck_sb[:, :, 512:], ck_d[:, :, 512:])
            nc.sync.dma_start(sk_sb[:, :, 512:], sk_d[:, :, 512:])
            nc.sync.dma_start(xt_sb[:, 2, :, :], xt_d[:, 2, :, :])
            nc.sync.dma_start(xt_sb[:, 3, :, :], xt_d[:, 3, :, :])
            nc.sync.dma_start(wo3_sb[:], wo3_d[:])

            ones_k = tabs.tile([P, 1], BF16, name="ones_k")
            nc.gpsimd.memset(ones_k[:], 1.0)
            ones_m = tabs.tile([1, P], BF16, name="ones_m")
            nc.gpsimd.memset(ones_m[:], 1.0)


            # persistent activation tiles
            q_re = {}
            q_im = {}
            k_re = {}
            k_im = {}
            for h in range(HPC):
                q_re[h] = qkp.tile([P, S], BF16, tag="q_re", name=f"q_re{h}")
                q_im[h] = qkp.tile([P, S], BF16, tag="q_im", name=f"q_im{h}")
                k_re[h] = qkp.tile([P, S], BF16, tag="k_re", name=f"k_re{h}")
                k_im[h] = qkp.tile([P, S], BF16, tag="k_im", name=f"k_im{h}")
            v_sb = big.tile([P, SQ, DPC], BF16, tag="vsb", name="v_sb")

            # ---------------- emission units ---------------------------
            def proj_q(n, h):
                ch = slice(512 * n, 512 * (n + 1))
                hsl = slice(P * h, P * (h + 1))
                pq = psp.tile([P, 512], F32, tag="pp", bufs=2, name="pq")
                for ko in range(KO):
                    nc.tensor.matmul(pq[:], wq_sb[:, ko, hsl],
                                     xt_sb[:, n, ko, :],
                                     start=(ko == 0), stop=(ko == KO - 1))
                # softplus(x) = ln(exp(x) + 1); Softplus has no ACT table
                # set in this build, Exp/Ln share one.
                eq = mup.tile([P, 512], BF16, tag="mu", name="eq")
                nc.scalar.activation(eq[:], pq[:], AF.Exp)
                mu = mup.tile([P, 512], BF16, tag="mu", name="mu_q")
                nc.scalar.activation(mu[:], eq[:], AF.Ln, bias=1.0)
                nc.vector.tensor_tensor(q_re[h][:, ch], mu[:],
                                        cos_sb[:, ch], ALU.mult)
                nc.vector.tensor_tensor(q_im[h][:, ch], mu[:],
                                        sin_sb[:, ch], ALU.mult)

            def proj_k(n, h):
                ch = slice(512 * n, 512 * (n + 1))
                hsl = slice(P * h, P * (h + 1))
                pk = psp.tile([P, 512], F32, tag="pp", bufs=2, name="pk")
                for ko in range(KO):
                    nc.tensor.matmul(pk[:], wk_sb[:, ko, hsl],
                                     xt_sb[:, n, ko, :],
                                     start=(ko == 0), stop=(ko == KO - 1))
                ek = mup.tile([P, 512], BF16, tag="mu", name="ek")
                nc.scalar.activation(ek[:], pk[:], AF.Exp)
                muk = mup.tile([P, 512], BF16, tag="mu", name="mu_k")
                nc.scalar.activation(muk[:], ek[:], AF.Ln, bias=1.0)
                nc.vector.tensor_tensor(k_re[h][:, ch], muk[:],
                                        ck_sb[:, h, ch], ALU.mult)
                nc.vector.tensor_tensor(k_im[h][:, ch], muk[:],
                                        sk_sb[:, h, ch], ALU.mult)

            def proj_v(i):
                # V for s'-tile i, both heads; [s', d] layout for A@V
                pv = psp.tile([P, DPC], F32, tag="pp", bufs=2, name="pv")
                for ko in range(KO):
                    nc.tensor.matmul(pv[:], xt_sb[:, i // 4, ko,
                                                  P * (i % 4):P * (i % 4) + P],
                                     wv_sb[:, ko, :],
                                     start=(ko == 0), stop=(ko == KO - 1))
                nc.vector.tensor_copy(out=v_sb[:, i, :], in_=pv[:])

            # ---- attention block for t-chunk j, head h. Part A = first
            # two score tiles (keeps the exp stream ahead of interleaved
            # projection ACTs), attn_B = rest + rowsum reciprocal; the
            # returned closure finishes normalize + gather-DMA and is
            # emitted after the NEXT block's part A so the PE never waits
            # on the ACT reciprocal chain.
            psum12 = {}

            def attn_A(j, h):
                st = {"j": j, "h": h, "tlo": 512 * j}
                tlo = st["tlo"]
                nlive = (tlo + 512) // P
                order = list(range(tlo // P, nlive)) + list(range(tlo // P))
                st["order"] = order
                st["nlive"] = nlive
                st["pav"] = psp.tile([P, 512], F32, tag="pav", bufs=2,
                                     name="pav")
                if j not in psum12:
                    psum12[j] = psp.tile([64, 512], F32, tag="prs",
                                         bufs=1, name="psum12")
                st["prs"] = psum12[j]
                st["pend"] = []
                st["idx"] = 0
                emit_tiles(st, 2)
                return st

            def emit_rs_av(st, e_t, lvs_t, i_t, first_t, last_t):
                h = st["h"]
                nc.tensor.matmul(st["prs"][32 * h:32 * h + 1, lvs_t],
                                 ones_k[:], e_t[:, lvs_t],
                                 start=first_t, stop=last_t)
                nc.tensor.matmul(st["pav"][:, lvs_t],
                                 v_sb[:, i_t, P * h:P * h + P],
                                 e_t[:, lvs_t],
                                 start=first_t, stop=last_t)

            def emit_tiles(st, count):
                h, tlo = st["h"], st["tlo"]
                order, nlive = st["order"], st["nlive"]
                pend = st["pend"]
                end = min(st["idx"] + count, nlive)
                for idx in range(st["idx"], end):
                    i = order[idx]
                    tshift = P * i - tlo
                    t0 = max(tshift, 0)
                    evs = slice(t0, 512)
                    # chunk 0 has no full tiles: keep full-range
                    # accumulation over zero-padded e there
                    lvs = slice(0, 512) if tlo == 0 else evs
                    ps = psp.tile([P, 512], F32, tag="ps", bufs=3,
                                  name="ps")
                    ksl = slice(P * i, P * (i + 1))
                    tvs = slice(tlo + t0, tlo + 512)
                    nc.tensor.matmul(ps[:, evs], k_re[h][:, ksl],
                                     q_re[h][:, tvs],
                                     start=True, stop=False)
                    nc.tensor.matmul(ps[:, evs], k_im[h][:, ksl],
                                     q_im[h][:, tvs],
                                     start=False, stop=True)
                    e = ep.tile([P, 512], BF16, tag="e", name="e")
                    if tlo == 0 and t0 > 0:
                        nc.vector.memset(e[:, 0:t0], 0.0)
                    nc.scalar.activation(e[:, evs], ps[:, evs], AF.Exp,
                                         scale=ISQ)
                    if tshift >= 0:
                        # multiplicative 0/1 causal triangle on the
                        # diagonal 128-col sub-block
                        nc.vector.tensor_tensor(e[:, t0:t0 + P],
                                                e[:, t0:t0 + P],
                                                tri_sb[:], ALU.mult)
                    pend.append((e, lvs, i, idx == 0, idx == nlive - 1))
                    if len(pend) > 2:
                        emit_rs_av(st, *pend.pop(0))
                st["idx"] = end

            def attn_B(st, cc_dma):
                h = st["h"]
                emit_tiles(st, st["nlive"])
                pend = st["pend"]
                st["pend"] = []
                for et in pend:
                    emit_rs_av(st, *et)
                # rec = 1/rowsum via exp(-ln(x)): same ACT table set as
                # the attention exps.
                lnt = smp.tile([1, 512], F32, tag="lnt", name="lnt")
                nc.scalar.activation(lnt[:], st["prs"][32 * h:32 * h + 1, :],
                                     AF.Ln)
                rec = smp.tile([1, 512], BF16, tag="rec", name="rec")
                nc.scalar.activation(rec[:], lnt[:], AF.Exp, scale=-1.0)

                def fin():
                    pb = psp.tile([P, 512], F32, tag="pp", bufs=2, name="pb")
                    nc.tensor.matmul(pb[:], ones_m[:], rec[:],
                                     start=True, stop=True)
                    bc = smp.tile([P, 512], BF16, tag="bc", name="bc")
                    nc.vector.tensor_copy(out=bc[:], in_=pb[:])
                    osb = obp.tile([P, 512], BF16, tag="osb", name="osb")
                    nc.vector.tensor_tensor(osb[:], st["pav"][:], bc[:],
                                            ALU.mult)
                    cc_dma(osb)
                return fin

            osbs = {}

            def stash(j, h):
                def f(osb):
                    osbs[(j, h)] = osb
                return f

            out3v = out3_d.rearrange("(a p) t -> p a t", p=P)

            def partial(j, last):
                """Per-core partial output projection for chunk j over its
                own 256 dims and ALL output columns; the host sums the 8
                cores' partials. No gather, no collectives: starts as soon
                as this chunk's attention normalize is done. Copies and
                DMAs alternate engines/queues; four m-slices batch into
                one DMA."""
                ch = slice(512 * j, 512 * (j + 1))
                tag = "ps" if last else "pp"
                bufs = 3 if last else 2
                grp = 1 if last else 4
                fo4 = None
                for m in range(NH * HD // P):
                    msl = slice(P * m, P * (m + 1))
                    po = psp.tile([P, 512], F32, tag=tag, bufs=bufs,
                                  name="pox")
                    for h in range(HPC):
                        nc.tensor.matmul(po[:], wo3_sb[:, h, msl],
                                         osbs[(j, h)][:],
                                         start=(h == 0), stop=(h == 1))
                    if m % grp == 0:
                        fo4 = obp.tile([P, grp, 512], BF16, tag="fo",
                                       name="fo4")
                    if m % 2 == 0:
                        nc.vector.tensor_copy(out=fo4[:, m % grp, :],
                                              in_=po[:])
                    else:
                        nc.scalar.activation(fo4[:, m % grp, :], po[:],
                                             AF.Copy)
                    # noqa: copy engine alternates with m
                    if m % grp == grp - 1:
                        dst = out3v[:, m - grp + 1:m + 1, ch]
                        if (m // grp) % 2 == 0:
                            nc.sync.dma_start(dst, fo4[:])
                        else:
                            nc.scalar.dma_start(dst, fo4[:])

            # ---------------- pipelined emission -----------------------
            proj_q(0, 0)
            proj_k(0, 0)
            proj_q(0, 1)
            proj_k(0, 1)
            for i in range(4):
                proj_v(i)

            stA = attn_A(0, 0)
            for j in range(NCH - 1):
                fin0 = attn_B(stA, stash(j, 0))
                stA = attn_A(j, 1)
                fin0()
                proj_q(j + 1, 0)
                fin1 = attn_B(stA, stash(j, 1))
                proj_k(j + 1, 0)
                fin1()
                proj_q(j + 1, 1)
                proj_k(j + 1, 1)
                for i in range(4 * (j + 1), 4 * (j + 1) + 4):
                    proj_v(i)
                partial(j, last=False)
                stA = attn_A(j + 1, 0)

            fin0 = attn_B(stA, stash(3, 0))
            stA = attn_A(3, 1)
            fin0()
            fin1 = attn_B(stA, stash(3, 1))
            fin1()
            partial(3, last=True)

    return nc


_NC_CACHE = None
_LAST_IN_MAPS = None


def _get_nc():
    global _NC_CACHE
    if _NC_CACHE is None:
        _NC_CACHE = build_nc()
    return _NC_CACHE


def kernel(hidden_states, wq, wk, wv, wo, learned_bias, attention_mask):
    bf16 = ml_dtypes.bfloat16
    x = np.asarray(hidden_states, dtype=np.float32).reshape(S, HID)
    # [p, n, ko, s] partition-major staging of X^T
    xt = np.ascontiguousarray(
        x.T.reshape(KO, P, NCH, 512).transpose(1, 2, 0, 3)).astype(bf16)

    wqT = np.asarray(wq, dtype=np.float32).T.astype(bf16)   # [HID, out]
    wkT = np.asarray(wk, dtype=np.float32).T.astype(bf16)
    wvT = np.asarray(wv, dtype=np.float32).T.astype(bf16)
    woT = np.asarray(wo, dtype=np.float32).T.astype(bf16)

    def wdev(w):
        # [HID, o] -> [p, ko, o]
        return np.ascontiguousarray(
            np.asarray(w).reshape(KO, P, -1).transpose(1, 0, 2))

    inv_freq = 1.0 / (BASE ** (np.arange(HD, dtype=np.float32) / HD))
    pos = np.arange(S, dtype=np.float32)
    freqs = pos[:, None] * inv_freq[None, :]                # [S, HD]
    cosT = np.ascontiguousarray(np.cos(freqs).T).astype(bf16)  # [HD, S]
    sinT = np.ascontiguousarray(np.sin(freqs).T).astype(bf16)

    bias = np.clip(np.asarray(learned_bias, dtype=np.float32),
                   -2.0 * math.pi, 0.0).reshape(NH, HD)     # [NH, HD]
    # k-side tables with bias folded: [HD, NH, S]
    kf = freqs[None, :, :] + bias[:, None, :]               # [NH, S, HD]
    ckT = np.ascontiguousarray(np.cos(kf).transpose(2, 0, 1)).astype(bf16)
    skT = np.ascontiguousarray(np.sin(kf).transpose(2, 0, 1)).astype(bf16)

    tri = (np.arange(P)[:, None] <= np.arange(P)[None, :]).astype(bf16)

    in_maps = []
    for c in range(N_CORES):
        osl = slice(DPC * c, DPC * (c + 1))
        heads = slice(HPC * c, HPC * (c + 1))
        in_maps.append({
            "xt": xt,
            "wq": wdev(wqT[:, osl]),
            "wk": wdev(wkT[:, osl]),
            "wv": wdev(wvT[:, osl]),
            "wo3": np.ascontiguousarray(
                woT[osl].reshape(HPC, P, NH * HD).transpose(1, 0, 2)),
            "cosT": cosT,
            "sinT": sinT,
            "ckT": np.ascontiguousarray(ckT[:, heads, :]),
            "skT": np.ascontiguousarray(skT[:, heads, :]),
            "tri": tri,
        })

    global _LAST_IN_MAPS
    _LAST_IN_MAPS = in_maps
    nc = _get_nc()
    res = run_bass_kernel_spmd(nc, in_maps, list(range(N_CORES)))
    finalT = np.sum([np.asarray(res.results[c]["out3"], dtype=np.float32)
                     for c in range(N_CORES)], axis=0)  # [HID, S]
    return np.ascontiguousarray(finalT.T)[None].astype(np.float32)


# revision 17
# speedup vs baseline: 1.0346x; 1.0346x over previous
"""PoPE attention Trainium2 kernel, 8-core tensor-parallel (2 heads/core).

Self-contained: hardcodes shapes B=1, S=2048, HID=2048, NH=16, HD=128.

Math (per reference):
  q/k/v = X @ w{q,k,v}.T, split into 16 heads of dim 128
  mu_{q,k} = softplus(q/k)
  q_polar = mu_q * (cos/sin)(pos*invfreq);  k uses angles + clipped bias
  scores  = (q_re.k_re + q_im.k_im)/sqrt(128) + causal_mask
  out     = softmax(scores) @ v;  final = out @ wo.T

Sharding: heads 2c,2c+1 on core c (wq/wk/wv column-sharded). The output
projection is row-sharded after on-device chunked AllGathers of the
per-core attention outputs; each core computes a 256-column slice of the
final output (transposed layout), gathered and transposed on host.

Device layout is feature-major ("transposed"): activations live as
[d, s] so every matmul contracts along partitions with zero on-device
transposes. Scores are computed as E[s', t] so softmax's sum reduction
is a ones-vector matmul and E feeds the A@V matmul directly.

All DRAM inputs are pre-arranged on the host into the exact SBUF layout
(partition-major) so every load is a contiguous DMA, and the startup
loads are spread across four engine queues so issue and transfer
overlap. The k-side rotation bias is folded into per-head cos/sin
tables on the host. The kernel is emitted as one software-pipelined
stream: projections for chunk j+1 and the output projection for chunk
j-1 are interleaved between the attention blocks of chunk j, each
block's first two score tiles lead the interleaved projection
activations on the ACT queue, and the normalize/gather tail of each
block is deferred past the next block's head so the PE never waits on
the reciprocal chain. The causal mask is a multiplicative 0/1 triangle
applied to E after the exp; diagonal-tile AV/rowsum matmuls run on
partial free ranges. The last chunk's AllGathers are per-head and its
output projection consumes the first head's gather while the second is
still in flight.
"""

import math
import sys
import types

import numpy as np
import ml_dtypes

import concourse.bass as bass
import concourse.mybir as mybir
import concourse.tile as tile
from concourse.bass_utils import run_bass_kernel_spmd

# ---------------------------------------------------------------- constants
B, S, HID = 1, 2048, 2048
NH, HD = 16, 128
BASE = 10000.0
N_CORES = 8
HPC = NH // N_CORES          # heads per core = 2
DPC = HPC * HD               # head dims per core = 256
P = 128                      # partitions
KO = HID // P                # 16 k-subtiles
NCH = S // 512               # 4 free-dim chunks of 512
SQ = S // P                  # 16 s'-tiles of 128
BF16 = mybir.dt.bfloat16
F32 = mybir.dt.float32
AF = mybir.ActivationFunctionType
ALU = mybir.AluOpType
ISQ = 1.0 / math.sqrt(HD)


def _install_ntff_hook():
    """Bare agent image lacks antenv.axon_hooks; synthesize it from the boot
    module's ctypes NTFF hook so run_bass_kernel_spmd(trace=True) works."""
    if "antenv.axon_hooks" in sys.modules:
        return
    try:
        from trn_agent_boot.trn_boot import _ntff_profile_via_ctypes
        hook = _ntff_profile_via_ctypes("/opt/axon/libaxon_pjrt.so")
    except Exception:
        hook = None
    mod = types.ModuleType("antenv.axon_hooks")
    mod.get_axon_ntff_profile_hook = lambda: hook
    mod.set_axon_ntff_profile_hook = lambda h: None
    sys.modules["antenv.axon_hooks"] = mod


_install_ntff_hook()

_TPB_ENGINES = (
    mybir.EngineType.PE,
    mybir.EngineType.Activation,
    mybir.EngineType.DVE,
    mybir.EngineType.Pool,
    mybir.EngineType.SP,
)


class SplitDrainTileContext(tile.TileContext):
    """This walrus build allows at most ONE sem wait per TPB instruction.
    Legalize: move extra waits onto single-wait NOPs emitted just before the
    instruction on the same engine, and split the tail drain the same way."""

    def _split_multiwait(self, insts):
        out = []
        for inst in insts:
            si = getattr(inst, "sync_info", None)
            if (
                si is not None
                and si.on_wait
                and len(si.on_wait) > 1
                and inst.engine in _TPB_ENGINES
            ):
                waits = list(si.on_wait)
                for w in waits[:-1]:
                    out.append(
                        mybir.InstNoOp(
                            name=self.nc.get_next_instruction_name(),
                            sync_info=mybir.SyncInfo(on_wait=[w], on_update=[]),
                            bass_nofuse=True,
                            engine=inst.engine,
                        )
                    )
                si.on_wait = waits[-1:]
            out.append(inst)
        return out

    def _lower_ordered_insts(self, ordered):
        for k in list(ordered.keys()):
            ordered[k] = self._split_multiwait(ordered[k])
        return super()._lower_ordered_insts(ordered)

    def _drain_and_barrier(self, tick_clock, wait_clock):
        from concourse.vector_clock import ScopedClock

        drain_inst = self.nc.sync.drain()
        wait_clock.add_sem_waits(
            drain_inst.ins, ScopedClock({None: tick_clock.global_clock})
        )
        waits = list(drain_inst.ins.sync_info.on_wait or [])
        if len(waits) > 1:
            drain_inst.ins.sync_info.on_wait = waits[:1]
            for w in waits[1:]:
                d2 = self.nc.sync.drain()
                if d2.ins.sync_info is None:
                    d2.ins.sync_info = mybir.SyncInfo(on_wait=[w], on_update=[])
                else:
                    d2.ins.sync_info.on_wait = [w]

        self.nc.all_engine_barrier()
        assert self.sems is not None
        popped = self.nc._tile_sem_poison_stack.pop()
        assert popped is self._sem_poison
        self.nc.clear_and_free_semaphores(list(self.sems.allocated().values()))
        self.nc.all_engine_barrier()


def build_nc():
    nc = bass.Bass("TRN2", target_bir_lowering=False, debug=False,
                   num_devices=N_CORES)

    # all inputs pre-arranged on host to partition-major device layout
    xt_d = nc.dram_tensor("xt", [P, NCH, KO, 512], BF16,
                          kind="ExternalInput").ap()
    wq_d = nc.dram_tensor("wq", [P, KO, DPC], BF16, kind="ExternalInput").ap()
    wk_d = nc.dram_tensor("wk", [P, KO, DPC], BF16, kind="ExternalInput").ap()
    wv_d = nc.dram_tensor("wv", [P, KO, DPC], BF16, kind="ExternalInput").ap()
    cos_d = nc.dram_tensor("cosT", [P, S], BF16, kind="ExternalInput").ap()
    sin_d = nc.dram_tensor("sinT", [P, S], BF16, kind="ExternalInput").ap()
    # per-head k-side tables with the clipped bias folded in
    ck_d = nc.dram_tensor("ckT", [P, HPC, S], BF16, kind="ExternalInput").ap()
    sk_d = nc.dram_tensor("skT", [P, HPC, S], BF16, kind="ExternalInput").ap()
    tri_d = nc.dram_tensor("tri", [P, P], BF16, kind="ExternalInput").ap()
    wo3_d = nc.dram_tensor("wo3", [P, HPC, NH * HD], BF16,
                           kind="ExternalInput").ap()
    out3_d = nc.dram_tensor("out3", [NH * HD, S], BF16,
                            kind="ExternalOutput").ap()

    with SplitDrainTileContext(nc) as tc:
        with tc.tile_pool(name="big", bufs=1) as big, \
             tc.tile_pool(name="wts", bufs=1) as wts, \
             tc.tile_pool(name="tabs", bufs=1) as tabs, \
             tc.tile_pool(name="qk", bufs=2) as qkp, \
             tc.tile_pool(name="mu", bufs=4) as mup, \
             tc.tile_pool(name="ep", bufs=6) as ep, \
             tc.tile_pool(name="sm", bufs=2) as smp, \
             tc.tile_pool(name="ob", bufs=3) as obp, \
             tc.tile_pool(name="ps", bufs=1, space="PSUM") as psp, \
             tc.tile_pool(name="dram", bufs=1, space="DRAM") as dram:

            # ------- loads: SP+ACT queues split the PE-critical path in
            # halves (a single DMA queue sustains only ~150GB/s); gpsimd
            # carries the DVE tables and everything needed later.
            wq_sb = wts.tile([P, KO, DPC], BF16, name="wq_sb")
            xt_sb = big.tile([P, NCH, KO, 512], BF16, tag="big", name="xt_sb")
            wk_sb = wts.tile([P, KO, DPC], BF16, name="wk_sb")
            wo3_sb = wts.tile([P, HPC, NH * HD], BF16, name="wo3_sb")
            nc.sync.dma_start(wq_sb[:, 0:4, :], wq_d[:, 0:4, :])
            nc.scalar.dma_start(xt_sb[:, 0, 0:2, :], xt_d[:, 0, 0:2, :])
            nc.sync.dma_start(xt_sb[:, 0, 2:4, :], xt_d[:, 0, 2:4, :])
            nc.scalar.dma_start(wq_sb[:, 4:8, :], wq_d[:, 4:8, :])
            nc.sync.dma_start(xt_sb[:, 0, 4:6, :], xt_d[:, 0, 4:6, :])
            nc.scalar.dma_start(xt_sb[:, 0, 6:8, :], xt_d[:, 0, 6:8, :])
            nc.sync.dma_start(wq_sb[:, 8:12, :], wq_d[:, 8:12, :])
            nc.scalar.dma_start(xt_sb[:, 0, 8:10, :], xt_d[:, 0, 8:10, :])
            nc.sync.dma_start(xt_sb[:, 0, 10:12, :], xt_d[:, 0, 10:12, :])
            nc.scalar.dma_start(wq_sb[:, 12:16, :], wq_d[:, 12:16, :])
            nc.sync.dma_start(xt_sb[:, 0, 12:14, :], xt_d[:, 0, 12:14, :])
            nc.scalar.dma_start(xt_sb[:, 0, 14:16, :], xt_d[:, 0, 14:16, :])
            nc.sync.dma_start(wk_sb[:, 0:8, :], wk_d[:, 0:8, :])
            nc.scalar.dma_start(wk_sb[:, 8:16, :], wk_d[:, 8:16, :])

            # gpsimd queue: tables for chunk 0 (DVE-critical), then wv
            cos_sb = tabs.tile([P, S], BF16, name="cos_sb")
            sin_sb = tabs.tile([P, S], BF16, name="sin_sb")
            ck_sb = tabs.tile([P, HPC, S], BF16, name="ck_sb")
            sk_sb = tabs.tile([P, HPC, S], BF16, name="sk_sb")
            tri_sb = tabs.tile([P, P], BF16, name="tri_sb")
            wv_sb = wts.tile([P, KO, DPC], BF16, name="wv_sb")
            nc.gpsimd.dma_start(cos_sb[:, 0:512], cos_d[:, 0:512])
            nc.gpsimd.dma_start(sin_sb[:, 0:512], sin_d[:, 0:512])
            nc.gpsimd.dma_start(ck_sb[:, :, 0:512], ck_d[:, :, 0:512])
            nc.gpsimd.dma_start(sk_sb[:, :, 0:512], sk_d[:, :, 0:512])
            nc.gpsimd.dma_start(tri_sb[:], tri_d[:])
            nc.gpsimd.dma_start(wv_sb[:], wv_d[:])

            # everything later-needed goes sequentially on SP so the ACT
            # and gpsimd queues stay clear for compute/memsets/collectives
            nc.sync.dma_start(xt_sb[:, 1, :, :], xt_d[:, 1, :, :])
            nc.sync.dma_start(cos_sb[:, 512:], cos_d[:, 512:])
            nc.sync.dma_start(sin_sb[:, 512:], sin_d[:, 512:])
            nc.sync.dma_start(ck_sb[:, :, 512:], ck_d[:, :, 512:])
            nc.sync.dma_start(sk_sb[:, :, 512:], sk_d[:, :, 512:])
            nc.sync.dma_start(xt_sb[:, 2, :, :], xt_d[:, 2, :, :])
            nc.sync.dma_start(xt_sb[:, 3, :, :], xt_d[:, 3, :, :])
            nc.sync.dma_start(wo3_sb[:], wo3_d[:])

            ones_k = tabs.tile([P, 1], BF16, name="ones_k")
            nc.gpsimd.memset(ones_k[:], 1.0)
            ones_m = tabs.tile([1, P], BF16, name="ones_m")
            nc.gpsimd.memset(ones_m[:], 1.0)


            # persistent activation tiles
            q_re = {}
            q_im = {}
            k_re = {}
            k_im = {}
            for h in range(HPC):
                q_re[h] = qkp.tile([P, S], BF16, tag="q_re", name=f"q_re{h}")
                q_im[h] = qkp.tile([P, S], BF16, tag="q_im", name=f"q_im{h}")
                k_re[h] = qkp.tile([P, S], BF16, tag="k_re", name=f"k_re{h}")
                k_im[h] = qkp.tile([P, S], BF16, tag="k_im", name=f"k_im{h}")
            v_sb = big.tile([P, SQ, DPC], BF16, tag="vsb", name="v_sb")

            # ---------------- emission units ---------------------------
            def proj_q(n, h):
                ch = slice(512 * n, 512 * (n + 1))
                hsl = slice(P * h, P * (h + 1))
                pq = psp.tile([P, 512], F32, tag="pp", bufs=2, name="pq")
                for ko in range(KO):
                    nc.tensor.matmul(pq[:], wq_sb[:, ko, hsl],
                                     xt_sb[:, n, ko, :],
                                     start=(ko == 0), stop=(ko == KO - 1))
                # softplus(x) = ln(exp(x) + 1); Softplus has no ACT table
                # set in this build, Exp/Ln share one.
                eq = mup.tile([P, 512], BF16, tag="mu", name="eq")
                nc.scalar.activation(eq[:], pq[:], AF.Exp)
                mu = mup.tile([P, 512], BF16, tag="mu", name="mu_q")
                nc.scalar.activation(mu[:], eq[:], AF.Ln, bias=1.0)
                nc.vector.tensor_tensor(q_re[h][:, ch], mu[:],
                                        cos_sb[:, ch], ALU.mult)
                nc.vector.tensor_tensor(q_im[h][:, ch], mu[:],
                                        sin_sb[:, ch], ALU.mult)

            def proj_k(n, h):
                ch = slice(512 * n, 512 * (n + 1))
                hsl = slice(P * h, P * (h + 1))
                pk = psp.tile([P, 512], F32, tag="pp", bufs=2, name="pk")
                for ko in range(KO):
                    nc.tensor.matmul(pk[:], wk_sb[:, ko, hsl],
                                     xt_sb[:, n, ko, :],
                                     start=(ko == 0), stop=(ko == KO - 1))
                ek = mup.tile([P, 512], BF16, tag="mu", name="ek")
                nc.scalar.activation(ek[:], pk[:], AF.Exp)
                muk = mup.tile([P, 512], BF16, tag="mu", name="mu_k")
                nc.scalar.activation(muk[:], ek[:], AF.Ln, bias=1.0)
                nc.vector.tensor_tensor(k_re[h][:, ch], muk[:],
                                        ck_sb[:, h, ch], ALU.mult)
                nc.vector.tensor_tensor(k_im[h][:, ch], muk[:],
                                        sk_sb[:, h, ch], ALU.mult)

            def proj_v(i):
                # V for s'-tile i, both heads; [s', d] layout for A@V
                pv = psp.tile([P, DPC], F32, tag="pp", bufs=2, name="pv")
                for ko in range(KO):
                    nc.tensor.matmul(pv[:], xt_sb[:, i // 4, ko,
                                                  P * (i % 4):P * (i % 4) + P],
                                     wv_sb[:, ko, :],
                                     start=(ko == 0), stop=(ko == KO - 1))
                nc.vector.tensor_copy(out=v_sb[:, i, :], in_=pv[:])

            # ---- attention block for t-chunk j, head h. Part A = first
            # two score tiles (keeps the exp stream ahead of interleaved
            # projection ACTs), attn_B = rest + rowsum reciprocal; the
            # returned closure finishes normalize + gather-DMA and is
            # emitted after the NEXT block's part A so the PE never waits
            # on the ACT reciprocal chain.
            psum12 = {}

            def attn_A(j, h):
                st = {"j": j, "h": h, "tlo": 512 * j}
                tlo = st["tlo"]
                nlive = (tlo + 512) // P
                order = list(range(tlo // P, nlive)) + list(range(tlo // P))
                st["order"] = order
                st["nlive"] = nlive
                st["pav"] = psp.tile([P, 512], F32, tag="pav", bufs=2,
                                     name="pav")
                if j not in psum12:
                    psum12[j] = psp.tile([64, 512], F32, tag="prs",
                                         bufs=1, name="psum12")
                st["prs"] = psum12[j]
                st["pend"] = []
                st["idx"] = 0
                emit_tiles(st, 2)
                return st

            def emit_rs_av(st, e_t, lvs_t, i_t, first_t, last_t):
                h = st["h"]
                nc.tensor.matmul(st["prs"][32 * h:32 * h + 1, lvs_t],
                                 ones_k[:], e_t[:, lvs_t],
                                 start=first_t, stop=last_t)
                nc.tensor.matmul(st["pav"][:, lvs_t],
                                 v_sb[:, i_t, P * h:P * h + P],
                                 e_t[:, lvs_t],
                                 start=first_t, stop=last_t)

            def emit_tiles(st, count):
                h, tlo = st["h"], st["tlo"]
                order, nlive = st["order"], st["nlive"]
                pend = st["pend"]
                end = min(st["idx"] + count, nlive)
                for idx in range(st["idx"], end):
                    i = order[idx]
                    tshift = P * i - tlo
                    t0 = max(tshift, 0)
                    evs = slice(t0, 512)
                    # chunk 0 has no full tiles: keep full-range
                    # accumulation over zero-padded e there
                    lvs = slice(0, 512) if tlo == 0 else evs
                    ps = psp.tile([P, 512], F32, tag="ps", bufs=3,
                                  name="ps")
                    ksl = slice(P * i, P * (i + 1))
                    tvs = slice(tlo + t0, tlo + 512)
                    nc.tensor.matmul(ps[:, evs], k_re[h][:, ksl],
                                     q_re[h][:, tvs],
                                     start=True, stop=False)
                    nc.tensor.matmul(ps[:, evs], k_im[h][:, ksl],
                                     q_im[h][:, tvs],
                                     start=False, stop=True)
                    e = ep.tile([P, 512], BF16, tag="e", name="e")
                    if tlo == 0 and t0 > 0:
                        nc.vector.memset(e[:, 0:t0], 0.0)
                    nc.scalar.activation(e[:, evs], ps[:, evs], AF.Exp,
                                         scale=ISQ)
                    if tshift >= 0:
                        # multiplicative 0/1 causal triangle on the
                        # diagonal 128-col sub-block
                        nc.vector.tensor_tensor(e[:, t0:t0 + P],
                                                e[:, t0:t0 + P],
                                                tri_sb[:], ALU.mult)
                    pend.append((e, lvs, i, idx == 0, idx == nlive - 1))
                    if len(pend) > 2:
                        emit_rs_av(st, *pend.pop(0))
                st["idx"] = end

            def attn_B(st, cc_dma):
                h = st["h"]
                emit_tiles(st, st["nlive"])
                pend = st["pend"]
                st["pend"] = []
                for et in pend:
                    emit_rs_av(st, *et)
                # rec = 1/rowsum via exp(-ln(x)): same ACT table set as
                # the attention exps.
                lnt = smp.tile([1, 512], F32, tag="lnt", name="lnt")
                nc.scalar.activation(lnt[:], st["prs"][32 * h:32 * h + 1, :],
                                     AF.Ln)
                rec = smp.tile([1, 512], BF16, tag="rec", name="rec")
                nc.scalar.activation(rec[:], lnt[:], AF.Exp, scale=-1.0)

                def fin():
                    pb = psp.tile([P, 512], F32, tag="pp", bufs=2, name="pb")
                    nc.tensor.matmul(pb[:], ones_m[:], rec[:],
                                     start=True, stop=True)
                    bc = smp.tile([P, 512], BF16, tag="bc", name="bc")
                    nc.vector.tensor_copy(out=bc[:], in_=pb[:])
                    osb = obp.tile([P, 512], BF16, tag="osb", name="osb")
                    nc.vector.tensor_tensor(osb[:], st["pav"][:], bc[:],
                                            ALU.mult)
                    cc_dma(osb)
                return fin

            osbs = {}

            def stash(j, h):
                def f(osb):
                    osbs[(j, h)] = osb
                return f

            out3v = out3_d.rearrange("(a p) t -> p a t", p=P)

            def partial(j, last):
                """Per-core partial output projection for chunk j over its
                own 256 dims and ALL output columns; the host sums the 8
                cores' partials. No gather, no collectives: starts as soon
                as this chunk's attention normalize is done. Copies and
                DMAs alternate engines/queues; four m-slices batch into
                one DMA."""
                ch = slice(512 * j, 512 * (j + 1))
                tag = "ps" if last else "pp"
                bufs = 3 if last else 2
                grp = 2 if last else 4
                fo4 = None
                for m in range(NH * HD // P):
                    msl = slice(P * m, P * (m + 1))
                    po = psp.tile([P, 512], F32, tag=tag, bufs=bufs,
                                  name="pox")
                    for h in range(HPC):
                        nc.tensor.matmul(po[:], wo3_sb[:, h, msl],
                                         osbs[(j, h)][:],
                                         start=(h == 0), stop=(h == 1))
                    if m % grp == 0:
                        fo4 = obp.tile([P, grp, 512], BF16, tag="fo",
                                       name="fo4")
                    if m % 2 == 0:
                        nc.vector.tensor_copy(out=fo4[:, m % grp, :],
                                              in_=po[:])
                    else:
                        nc.scalar.activation(fo4[:, m % grp, :], po[:],
                                             AF.Copy)
                    if m % grp == grp - 1:
                        dst = out3v[:, m - grp + 1:m + 1, ch]
                        if (m // grp) % 2 == 0:
                            nc.sync.dma_start(dst, fo4[:])
                        else:
                            nc.scalar.dma_start(dst, fo4[:])

            # ---------------- pipelined emission -----------------------
            proj_q(0, 0)
            proj_k(0, 0)
            proj_q(0, 1)
            proj_k(0, 1)
            for i in range(4):
                proj_v(i)

            stA = attn_A(0, 0)
            for j in range(NCH - 1):
                fin0 = attn_B(stA, stash(j, 0))
                stA = attn_A(j, 1)
                fin0()
                proj_q(j + 1, 0)
                fin1 = attn_B(stA, stash(j, 1))
                proj_k(j + 1, 0)
                fin1()
                proj_q(j + 1, 1)
                proj_k(j + 1, 1)
                for i in range(4 * (j + 1), 4 * (j + 1) + 4):
                    proj_v(i)
                partial(j, last=False)
                stA = attn_A(j + 1, 0)

            fin0 = attn_B(stA, stash(3, 0))
            stA = attn_A(3, 1)
            fin0()
            fin1 = attn_B(stA, stash(3, 1))
            fin1()
            partial(3, last=True)

    return nc


_NC_CACHE = None
_LAST_IN_MAPS = None


def _get_nc():
    global _NC_CACHE
    if _NC_CACHE is None:
        _NC_CACHE = build_nc()
    return _NC_CACHE


def kernel(hidden_states, wq, wk, wv, wo, learned_bias, attention_mask):
    bf16 = ml_dtypes.bfloat16
    x = np.asarray(hidden_states, dtype=np.float32).reshape(S, HID)
    # [p, n, ko, s] partition-major staging of X^T
    xt = np.ascontiguousarray(
        x.T.reshape(KO, P, NCH, 512).transpose(1, 2, 0, 3)).astype(bf16)

    wqT = np.asarray(wq, dtype=np.float32).T.astype(bf16)   # [HID, out]
    wkT = np.asarray(wk, dtype=np.float32).T.astype(bf16)
    wvT = np.asarray(wv, dtype=np.float32).T.astype(bf16)
    woT = np.asarray(wo, dtype=np.float32).T.astype(bf16)

    def wdev(w):
        # [HID, o] -> [p, ko, o]
        return np.ascontiguousarray(
            np.asarray(w).reshape(KO, P, -1).transpose(1, 0, 2))

    inv_freq = 1.0 / (BASE ** (np.arange(HD, dtype=np.float32) / HD))
    pos = np.arange(S, dtype=np.float32)
    freqs = pos[:, None] * inv_freq[None, :]                # [S, HD]
    cosT = np.ascontiguousarray(np.cos(freqs).T).astype(bf16)  # [HD, S]
    sinT = np.ascontiguousarray(np.sin(freqs).T).astype(bf16)

    bias = np.clip(np.asarray(learned_bias, dtype=np.float32),
                   -2.0 * math.pi, 0.0).reshape(NH, HD)     # [NH, HD]
    # k-side tables with bias folded: [HD, NH, S]
    kf = freqs[None, :, :] + bias[:, None, :]               # [NH, S, HD]
    ckT = np.ascontiguousarray(np.cos(kf).transpose(2, 0, 1)).astype(bf16)
    skT = np.ascontiguousarray(np.sin(kf).transpose(2, 0, 1)).astype(bf16)

    tri = (np.arange(P)[:, None] <= np.arange(P)[None, :]).astype(bf16)

    in_maps = []
    for c in range(N_CORES):
        osl = slice(DPC * c, DPC * (c + 1))
        heads = slice(HPC * c, HPC * (c + 1))
        in_maps.append({
            "xt": xt,
            "wq": wdev(wqT[:, osl]),
            "wk": wdev(wkT[:, osl]),
            "wv": wdev(wvT[:, osl]),
            "wo3": np.ascontiguousarray(
                woT[osl].reshape(HPC, P, NH * HD).transpose(1, 0, 2)),
            "cosT": cosT,
            "sinT": sinT,
            "ckT": np.ascontiguousarray(ckT[:, heads, :]),
            "skT": np.ascontiguousarray(skT[:, heads, :]),
            "tri": tri,
        })

    global _LAST_IN_MAPS
    _LAST_IN_MAPS = in_maps
    nc = _get_nc()
    res = run_bass_kernel_spmd(nc, in_maps, list(range(N_CORES)))
    finalT = np.sum([np.asarray(res.results[c]["out3"], dtype=np.float32)
                     for c in range(N_CORES)], axis=0)  # [HID, S]
    return np.ascontiguousarray(finalT.T)[None].astype(np.float32)
